# revision 2
# baseline (speedup 1.0000x reference)
import os
import sys

sys.path.insert(0, "/opt/trn_rl_repo")

import numpy as np

import concourse.bass as bass
import concourse.mybir as mybir
import concourse.tile as tile
from concourse import bacc

# ---------------- problem constants (hardcoded per spec) ----------------
N_NODES = 100000
N_EDGES = 640000
C = 128           # channels (in == out)
P = 128           # partitions
N_CORES = 8
NPC = N_NODES // N_CORES          # 12500 nodes per core
NBLK = (NPC + P - 1) // P         # 98 blocks per core
NPC_PAD = NBLK * P                # 12544

F32 = mybir.dt.float32
BF16 = mybir.dt.bfloat16


def _build_program(dchs):
    """Single launch. The host ships a slot-ordered, channel-major message
    stream msgs[c, idx(b,m,j)] = D[row]*vals_e*D[col]*X[col_e][c] (bf16).
    Row (block b, lane m) owns slots j < dchs[b]. Per block:
      red[c, m] = sum_j msgs          (DVE segmented reduce, innermost axis)
      ps[m, o]  = red.T @ W           (PE matmul, lhsT = red)
      y[b*128+m] = ps + bias          (DVE add, bf16 out)
    """
    dchs = [int(d) for d in dchs]
    s_total = sum(dchs)
    dmax = max(dchs)
    tot = 128 * s_total

    nc = bacc.Bacc("TRN2", target_bir_lowering=False, debug=False,
                   num_devices=N_CORES)
    msgs_d = nc.dram_tensor("msgs", [P, tot], BF16, kind="ExternalInput")
    wmat_d = nc.dram_tensor("wmat", [C, C], F32, kind="ExternalInput")
    biasb_d = nc.dram_tensor("biasb", [P, C], F32, kind="ExternalInput")
    y_d = nc.dram_tensor("y", [NPC_PAD, C], BF16, kind="ExternalOutput")

    with tile.TileContext(nc) as tc:
        with tc.tile_pool(name="consts", bufs=1) as consts, \
             tc.tile_pool(name="mpool", bufs=4) as mpool, \
             tc.tile_pool(name="rpool", bufs=4) as rpool, \
             tc.tile_pool(name="opool", bufs=4) as opool, \
             tc.tile_pool(name="ppool", bufs=4, space="PSUM") as ppool:

            wmat_t = consts.tile([C, C], F32)
            nc.sync.dma_start(out=wmat_t[:], in_=wmat_d[:])
            biasb_t = consts.tile([P, C], F32)
            nc.sync.dma_start(out=biasb_t[:], in_=biasb_d[:])

            off = 0
            for b, dch in enumerate(dchs):
                sz = P * dch
                mt = mpool.tile([P, P * dmax], BF16, name="mt", tag="mt")
                nc.sync.dma_start(out=mt[:, :sz], in_=msgs_d[:, off:off + sz])
                red = rpool.tile([P, P], F32, name="red")
                nc.vector.tensor_reduce(
                    out=red[:],
                    in_=mt[:, :sz].rearrange("p (m j) -> p m j", j=dch),
                    axis=mybir.AxisListType.X,
                    op=mybir.AluOpType.add,
                )
                ps = ppool.tile([P, C], F32, name="ps")
                nc.tensor.matmul(out=ps[:], lhsT=red[:], rhs=wmat_t[:],
                                 start=True, stop=True)
                yo = opool.tile([P, C], BF16, name="yo")
                nc.vector.tensor_tensor(out=yo[:], in0=ps[:], in1=biasb_t[:],
                                        op=mybir.AluOpType.add)
                nc.sync.dma_start(out=y_d[b * P:(b + 1) * P, :], in_=yo[:])
                off += sz
    nc.compile()
    return nc


def _preprocess(row, col, vals):
    """Host-side routing. Returns the global per-block slot budgets plus the
    per-core (edge slot index, row permutation) needed to build the message
    stream and unpermute the output."""
    row = np.asarray(row).astype(np.int64)
    col = np.asarray(col).astype(np.int64)
    vals = np.asarray(vals).astype(np.float32)

    deg = np.bincount(row, weights=vals.astype(np.float64),
                      minlength=N_NODES)
    D = (1.0 / np.sqrt(deg + 1.0)).astype(np.float32)
    # fold both normalizations into the per-edge weight
    w = D[row] * vals * D[col]

    owner = row // NPC
    cores = []
    dch_all = np.zeros((N_CORES, NBLK), np.int64)
    for c in range(N_CORES):
        m = owner == c
        r_loc = (row[m] - c * NPC).astype(np.int64)
        indeg = np.bincount(r_loc, minlength=NPC)
        order = np.argsort(-indeg, kind="stable")    # rank -> local row
        spad = np.pad(indeg[order], (0, NPC_PAD - NPC))
        dch_all[c] = spad.reshape(NBLK, P).max(1)
        cores.append((m, r_loc, order, indeg))

    dchs = np.maximum(dch_all.max(0), 1)
    off_b = np.concatenate([[0], np.cumsum(P * dchs)])[:-1]  # [NBLK]

    slot_idx = []          # per core: slot index per (its) edge
    orders = []
    for c in range(N_CORES):
        m, r_loc, order, indeg = cores[c]
        rank_of = np.empty(NPC, np.int64)
        rank_of[order] = np.arange(NPC)
        blk = rank_of // P
        lane = rank_of % P
        # ordinal j of each edge within its row
        o_e = np.argsort(r_loc, kind="stable")
        r_s = r_loc[o_e]
        starts = np.searchsorted(r_s, np.arange(NPC))
        offs = np.arange(len(r_s)) - starts[r_s]
        idx_sorted = off_b[blk[r_s]] + lane[r_s] * dchs[blk[r_s]] + offs
        idx = np.empty(len(r_s), np.int64)
        idx[o_e] = idx_sorted
        slot_idx.append(idx)
        orders.append(order)

    return dchs, off_b, slot_idx, orders, w, owner


_CACHE = {}


def _get_program(dchs):
    key = tuple(int(d) for d in dchs)
    if key not in _CACHE:
        _CACHE[key] = _build_program(dchs)
    return _CACHE[key]


def _run(nc, in_maps):
    if os.environ.get("KERNEL_SIM"):
        from concourse import bass_interp
        sim = bass_interp.MultiCoreSim(nc, N_CORES)
        for c in range(N_CORES):
            for k, v in in_maps[c].items():
                sim.cores[c].tensor(k)[:] = v
        sim.simulate()
        out_names = [
            a.memorylocations[0].name
            for a in nc.m.functions[0].allocations
            if isinstance(a, mybir.MemoryLocationSet)
            and a.kind == "ExternalOutput"
        ]
        return [{n: np.array(sim.cores[c].mem_tensor(n)) for n in out_names}
                for c in range(N_CORES)]
    from concourse.bass_utils import run_bass_kernel_spmd
    try:
        res = run_bass_kernel_spmd(nc, in_maps, core_ids=list(range(N_CORES)))
    except Exception:
        import time
        time.sleep(2.0)  # transient NRT/axon failures recover on retry
        res = run_bass_kernel_spmd(nc, in_maps, core_ids=list(range(N_CORES)))
    return res.results


def kernel(row, col, vals, X, weights, bias):
    import ml_dtypes

    X = np.asarray(X).astype(np.float32)
    weights = np.asarray(weights).astype(np.float32)
    bias = np.asarray(bias).astype(np.float32)
    col = np.asarray(col).astype(np.int64)

    dchs, off_b, slot_idx, orders, w, owner = _preprocess(row, col, vals)
    nc = _get_program(dchs)

    tot = int(P * dchs.sum())
    biasb = np.ascontiguousarray(np.broadcast_to(bias, (P, C)))

    in_maps = []
    for c in range(N_CORES):
        m = owner == c
        msgs = np.zeros((tot, C), np.float32)
        msgs[slot_idx[c]] = X[col[m]] * w[m][:, None]
        msgsT = np.ascontiguousarray(msgs.T).astype(ml_dtypes.bfloat16)
        in_maps.append({"msgs": msgsT, "wmat": weights, "biasb": biasb})

    res = _run(nc, in_maps)

    out = np.empty((N_NODES, C), np.float32)
    for c in range(N_CORES):
        y = np.asarray(res[c]["y"]).astype(np.float32)   # [NPC_PAD, C]
        out[c * NPC + orders[c]] = y[:NPC]
    return out


# revision 3
# speedup vs baseline: 1.5058x; 1.5058x over previous
import os
import sys

sys.path.insert(0, "/opt/trn_rl_repo")

import numpy as np

import concourse.bass as bass
import concourse.mybir as mybir
import concourse.tile as tile
from concourse import bacc

# ---------------- problem constants (hardcoded per spec) ----------------
N_NODES = 100000
N_EDGES = 640000
C = 128           # channels (in == out)
P = 128           # partitions
N_CORES = 8
NPC = N_NODES // N_CORES          # 12500 nodes per core
NBLK = (NPC + P - 1) // P         # 98 blocks per core
NPC_PAD = NBLK * P                # 12544
MAX_RUN_SLOTS = 12288             # cap per-DMA tile at 24KB/partition (bf16)

F32 = mybir.dt.float32
BF16 = mybir.dt.bfloat16


def _make_runs(dchs):
    """Split blocks into runs of consecutive equal-dch blocks, capped so one
    run's slots fit a reasonable SBUF tile. Returns (dch, b0, nblocks)."""
    runs = []
    b = 0
    while b < len(dchs):
        d = int(dchs[b])
        e = b
        slots = 0
        while e < len(dchs) and int(dchs[e]) == d \
                and slots + P * d <= MAX_RUN_SLOTS:
            slots += P * d
            e += 1
        runs.append((d, b, e - b))
        b = e
    return runs


def _build_program(dchs):
    """Single launch, transposed pipeline. Host ships channel-major messages
    msgs[c, idx(b,m,j)] = D[row]*vals_e*D[col]*X[col_e][c] (bf16), where row
    (block b, lane m) owns slots j < dchs[b]. Per equal-dch run of blocks:
      one DMA in, one segmented reduce  red[c, (m over run)] = sum_j msgs
    Per block:
      ps[o, m] = W.T @ red_block + bias (x) ones  (PE, bias as rank-1 matmul)
      yo[o, m] = copy(ps) bf16                    (Act engine)
    Per run: one DMA out into y[C, NPC_PAD] (channel-major)."""
    dchs = [int(d) for d in dchs]
    s_total = sum(dchs)
    tot = P * s_total
    runs = _make_runs(dchs)
    max_slots = max(P * d * n for d, _, n in runs)
    max_rb = max(n for _, _, n in runs)

    nc = bacc.Bacc("TRN2", target_bir_lowering=False, debug=False,
                   num_devices=N_CORES)
    msgs_d = nc.dram_tensor("msgs", [P, tot], BF16, kind="ExternalInput")
    wmat_d = nc.dram_tensor("wmat", [C, C], BF16, kind="ExternalInput")
    biasr_d = nc.dram_tensor("biasr", [1, C], BF16, kind="ExternalInput")
    y_d = nc.dram_tensor("y", [C, NPC_PAD], BF16, kind="ExternalOutput")

    with tile.TileContext(nc) as tc:
        with tc.tile_pool(name="consts", bufs=1) as consts, \
             tc.tile_pool(name="mpool", bufs=3) as mpool, \
             tc.tile_pool(name="rpool", bufs=3) as rpool, \
             tc.tile_pool(name="opool", bufs=3) as opool, \
             tc.tile_pool(name="ppool", bufs=4, space="PSUM") as ppool:

            wmat_t = consts.tile([C, C], BF16)
            nc.sync.dma_start(out=wmat_t[:], in_=wmat_d[:])
            biasr_t = consts.tile([1, C], BF16)
            nc.sync.dma_start(out=biasr_t[:], in_=biasr_d[:])
            ones_t = consts.tile([1, P], BF16)
            nc.vector.memset(ones_t[:], 1.0)

            off = 0
            for dch, b0, rb in runs:
                sz = P * dch * rb
                mt = mpool.tile([P, max_slots], BF16, name="mt", tag="mt")
                nc.sync.dma_start(out=mt[:, :sz], in_=msgs_d[:, off:off + sz])
                red = rpool.tile([P, P * max_rb], BF16, name="red", tag="red")
                with nc.allow_low_precision(
                        reason="bf16 reduce writeout; inputs are bf16 "
                               "messages with ~0.4% noise already"):
                    nc.vector.tensor_reduce(
                        out=red[:, :P * rb],
                        in_=mt[:, :sz].rearrange("p (m j) -> p m j", j=dch),
                        axis=mybir.AxisListType.X,
                        op=mybir.AluOpType.add,
                    )
                yo = opool.tile([P, P * max_rb], BF16, name="yo", tag="yo")
                for bi in range(rb):
                    ps = ppool.tile([P, P], F32, name="ps")
                    nc.tensor.matmul(out=ps[:], lhsT=biasr_t[:],
                                     rhs=ones_t[:], start=True, stop=False)
                    nc.tensor.matmul(out=ps[:], lhsT=wmat_t[:],
                                     rhs=red[:, bi * P:(bi + 1) * P],
                                     start=False, stop=True)
                    nc.scalar.activation(
                        out=yo[:, bi * P:(bi + 1) * P], in_=ps[:],
                        func=mybir.ActivationFunctionType.Copy)
                nc.sync.dma_start(
                    out=y_d[:, (b0 * P):(b0 + rb) * P],
                    in_=yo[:, :rb * P])
                off += sz
    nc.compile()
    return nc


def _preprocess(row, col, vals):
    """Host-side routing. Returns the global per-block slot budgets plus the
    per-core (edge slot index, row permutation) needed to build the message
    stream and unpermute the output."""
    row = np.asarray(row).astype(np.int64)
    col = np.asarray(col).astype(np.int64)
    vals = np.asarray(vals).astype(np.float32)

    deg = np.bincount(row, weights=vals.astype(np.float64),
                      minlength=N_NODES)
    D = (1.0 / np.sqrt(deg + 1.0)).astype(np.float32)
    # fold both normalizations into the per-edge weight
    w = D[row] * vals * D[col]

    owner = row // NPC
    cores = []
    dch_all = np.zeros((N_CORES, NBLK), np.int64)
    for c in range(N_CORES):
        m = owner == c
        r_loc = (row[m] - c * NPC).astype(np.int64)
        indeg = np.bincount(r_loc, minlength=NPC)
        order = np.argsort(-indeg, kind="stable")    # rank -> local row
        spad = np.pad(indeg[order], (0, NPC_PAD - NPC))
        dch_all[c] = spad.reshape(NBLK, P).max(1)
        cores.append((m, r_loc, order, indeg))

    dchs = np.maximum(dch_all.max(0), 1)
    off_b = np.concatenate([[0], np.cumsum(P * dchs)])[:-1]  # [NBLK]

    slot_idx = []          # per core: slot index per (its) edge
    orders = []
    for c in range(N_CORES):
        m, r_loc, order, indeg = cores[c]
        rank_of = np.empty(NPC, np.int64)
        rank_of[order] = np.arange(NPC)
        blk = rank_of // P
        lane = rank_of % P
        # ordinal j of each edge within its row
        o_e = np.argsort(r_loc, kind="stable")
        r_s = r_loc[o_e]
        starts = np.searchsorted(r_s, np.arange(NPC))
        offs = np.arange(len(r_s)) - starts[r_s]
        idx_sorted = off_b[blk[r_s]] + lane[r_s] * dchs[blk[r_s]] + offs
        idx = np.empty(len(r_s), np.int64)
        idx[o_e] = idx_sorted
        slot_idx.append(idx)
        orders.append(order)

    return dchs, off_b, slot_idx, orders, w, owner


_CACHE = {}


def _get_program(dchs):
    key = tuple(int(d) for d in dchs)
    if key not in _CACHE:
        _CACHE[key] = _build_program(dchs)
    return _CACHE[key]


def _run(nc, in_maps):
    if os.environ.get("KERNEL_SIM"):
        from concourse import bass_interp
        sim = bass_interp.MultiCoreSim(nc, N_CORES)
        for c in range(N_CORES):
            for k, v in in_maps[c].items():
                sim.cores[c].tensor(k)[:] = v
        sim.simulate()
        out_names = [
            a.memorylocations[0].name
            for a in nc.m.functions[0].allocations
            if isinstance(a, mybir.MemoryLocationSet)
            and a.kind == "ExternalOutput"
        ]
        return [{n: np.array(sim.cores[c].mem_tensor(n)) for n in out_names}
                for c in range(N_CORES)]
    from concourse.bass_utils import run_bass_kernel_spmd
    try:
        res = run_bass_kernel_spmd(nc, in_maps, core_ids=list(range(N_CORES)))
    except Exception:
        import time
        time.sleep(2.0)  # transient NRT/axon failures recover on retry
        res = run_bass_kernel_spmd(nc, in_maps, core_ids=list(range(N_CORES)))
    return res.results


def kernel(row, col, vals, X, weights, bias):
    import ml_dtypes

    X = np.asarray(X).astype(np.float32)
    weights = np.asarray(weights).astype(np.float32)
    bias = np.asarray(bias).astype(np.float32)
    col = np.asarray(col).astype(np.int64)

    dchs, off_b, slot_idx, orders, w, owner = _preprocess(row, col, vals)
    nc = _get_program(dchs)

    tot = int(P * dchs.sum())
    wmat_bf = weights.astype(ml_dtypes.bfloat16)
    biasr = bias.reshape(1, C).astype(ml_dtypes.bfloat16)

    in_maps = []
    for c in range(N_CORES):
        m = owner == c
        msgs = np.zeros((tot, C), np.float32)
        msgs[slot_idx[c]] = X[col[m]] * w[m][:, None]
        msgsT = np.ascontiguousarray(msgs.T).astype(ml_dtypes.bfloat16)
        in_maps.append({"msgs": msgsT, "wmat": wmat_bf, "biasr": biasr})

    res = _run(nc, in_maps)

    out = np.empty((N_NODES, C), np.float32)
    for c in range(N_CORES):
        y = np.asarray(res[c]["y"]).astype(np.float32)   # [C, NPC_PAD]
        out[c * NPC + orders[c]] = y.T[:NPC]
    return out


# revision 6
# speedup vs baseline: 1.7025x; 1.1306x over previous
import os
import sys

sys.path.insert(0, "/opt/trn_rl_repo")

import numpy as np

import concourse.bass as bass
import concourse.mybir as mybir
import concourse.tile as tile
from concourse import bacc

# ---------------- problem constants (hardcoded per spec) ----------------
N_NODES = 100000
N_EDGES = 640000
C = 128           # channels (in == out)
P = 128           # partitions
N_CORES = 8
NPC = N_NODES // N_CORES          # 12500 nodes per core
NBLK = (NPC + P - 1) // P         # 98 blocks per core
NPC_PAD = NBLK * P                # 12544
MAX_RUN_SLOTS = 12288             # cap per-DMA tile at 24KB/partition (bf16)

F32 = mybir.dt.float32
BF16 = mybir.dt.bfloat16


def _make_runs(dchs):
    """Split blocks into runs of consecutive equal-dch blocks, capped so one
    run's slots fit a reasonable SBUF tile. Returns (dch, b0, nblocks)."""
    runs = []
    b = 0
    while b < len(dchs):
        d = int(dchs[b])
        e = b
        slots = 0
        while e < len(dchs) and int(dchs[e]) == d \
                and slots + P * d <= MAX_RUN_SLOTS:
            slots += P * d
            e += 1
        runs.append((d, b, e - b))
        b = e
    return runs


def _build_program(dchs):
    """Single launch, transposed pipeline. Host ships channel-major messages
    msgs[c, idx(b,j,m)] = D[row]*vals_e*D[col]*X[col_e][c] (bf16), where row
    (block b, lane m) owns slots j < dchs[b]; within a block slots are
    j-major so each j is a contiguous 128-column slab. Per block:
      ps[o, m] = sum_j W.T @ msg_j       (PE, segmented sum via f32 PSUM
                                          accumulation across j matmuls)
      yo[o, m] = Identity(ps + bias[o])  (Act engine, per-partition bias)
    One DMA in and one DMA out per equal-dch run of blocks."""
    dchs = [int(d) for d in dchs]
    s_total = sum(dchs)
    tot = P * s_total
    runs = _make_runs(dchs)
    max_slots = max(P * d * n for d, _, n in runs)
    max_rb = max(n for _, _, n in runs)

    nc = bacc.Bacc("TRN2", target_bir_lowering=False, debug=False,
                   num_devices=N_CORES)
    msgs_d = nc.dram_tensor("msgs", [P, tot], BF16, kind="ExternalInput")
    wmat_d = nc.dram_tensor("wmat", [C, C], BF16, kind="ExternalInput")
    biasr_d = nc.dram_tensor("biasr", [C, 1], F32, kind="ExternalInput")
    y_d = nc.dram_tensor("y", [C, NPC_PAD], BF16, kind="ExternalOutput")

    with tile.TileContext(nc) as tc:
        with tc.tile_pool(name="consts", bufs=1) as consts, \
             tc.tile_pool(name="mpool", bufs=3) as mpool, \
             tc.tile_pool(name="opool", bufs=3) as opool, \
             tc.tile_pool(name="ppool", bufs=4, space="PSUM") as ppool:

            wmat_t = consts.tile([C, C], BF16)
            nc.sync.dma_start(out=wmat_t[:], in_=wmat_d[:])
            biasr_t = consts.tile([C, 1], F32)
            nc.sync.dma_start(out=biasr_t[:], in_=biasr_d[:])

            off = 0
            for dch, b0, rb in runs:
                sz = P * dch * rb
                mt = mpool.tile([P, max_slots], BF16, name="mt", tag="mt")
                nc.sync.dma_start(out=mt[:, :sz], in_=msgs_d[:, off:off + sz])
                yo = opool.tile([P, P * max_rb], BF16, name="yo", tag="yo")
                for bi in range(rb):
                    ps = ppool.tile([P, P], F32, name="ps")
                    base = bi * dch * P
                    for j in range(dch):
                        nc.tensor.matmul(
                            out=ps[:], lhsT=wmat_t[:],
                            rhs=mt[:, base + j * P:base + (j + 1) * P],
                            start=(j == 0), stop=(j == dch - 1))
                    nc.scalar.activation(
                        out=yo[:, bi * P:(bi + 1) * P], in_=ps[:],
                        func=mybir.ActivationFunctionType.Identity,
                        bias=biasr_t[:, 0:1])
                nc.sync.dma_start(
                    out=y_d[:, (b0 * P):(b0 + rb) * P],
                    in_=yo[:, :rb * P])
                off += sz
    nc.compile()
    return nc


def _preprocess(row, col, vals):
    """Host-side routing. Returns the global per-block slot budgets plus the
    per-core (edge slot index, row permutation) needed to build the message
    stream and unpermute the output."""
    row = np.asarray(row).astype(np.int64)
    col = np.asarray(col).astype(np.int64)
    vals = np.asarray(vals).astype(np.float32)

    deg = np.bincount(row, weights=vals.astype(np.float64),
                      minlength=N_NODES)
    D = (1.0 / np.sqrt(deg + 1.0)).astype(np.float32)
    # fold both normalizations into the per-edge weight
    w = D[row] * vals * D[col]

    owner = row // NPC
    cores = []
    dch_all = np.zeros((N_CORES, NBLK), np.int64)
    for c in range(N_CORES):
        m = owner == c
        r_loc = (row[m] - c * NPC).astype(np.int64)
        indeg = np.bincount(r_loc, minlength=NPC)
        order = np.argsort(-indeg, kind="stable")    # rank -> local row
        spad = np.pad(indeg[order], (0, NPC_PAD - NPC))
        dch_all[c] = spad.reshape(NBLK, P).max(1)
        cores.append((m, r_loc, order, indeg))

    dchs = np.maximum(dch_all.max(0), 1)
    off_b = np.concatenate([[0], np.cumsum(P * dchs)])[:-1]  # [NBLK]

    slot_idx = []          # per core: slot index per (its) edge
    orders = []
    for c in range(N_CORES):
        m, r_loc, order, indeg = cores[c]
        rank_of = np.empty(NPC, np.int64)
        rank_of[order] = np.arange(NPC)
        blk = rank_of // P
        lane = rank_of % P
        # ordinal j of each edge within its row
        o_e = np.argsort(r_loc, kind="stable")
        r_s = r_loc[o_e]
        starts = np.searchsorted(r_s, np.arange(NPC))
        offs = np.arange(len(r_s)) - starts[r_s]
        idx_sorted = off_b[blk[r_s]] + offs * P + lane[r_s]
        idx = np.empty(len(r_s), np.int64)
        idx[o_e] = idx_sorted
        slot_idx.append(idx)
        orders.append(order)

    return dchs, off_b, slot_idx, orders, w, owner


_CACHE = {}


def _get_program(dchs):
    key = tuple(int(d) for d in dchs)
    if key not in _CACHE:
        _CACHE[key] = _build_program(dchs)
    return _CACHE[key]


def _run(nc, in_maps):
    if os.environ.get("KERNEL_SIM"):
        from concourse import bass_interp
        sim = bass_interp.MultiCoreSim(nc, N_CORES)
        for c in range(N_CORES):
            for k, v in in_maps[c].items():
                sim.cores[c].tensor(k)[:] = v
        sim.simulate()
        out_names = [
            a.memorylocations[0].name
            for a in nc.m.functions[0].allocations
            if isinstance(a, mybir.MemoryLocationSet)
            and a.kind == "ExternalOutput"
        ]
        return [{n: np.array(sim.cores[c].mem_tensor(n)) for n in out_names}
                for c in range(N_CORES)]
    from concourse.bass_utils import run_bass_kernel_spmd
    try:
        res = run_bass_kernel_spmd(nc, in_maps, core_ids=list(range(N_CORES)))
    except Exception:
        import time
        time.sleep(2.0)  # transient NRT/axon failures recover on retry
        res = run_bass_kernel_spmd(nc, in_maps, core_ids=list(range(N_CORES)))
    return res.results


def kernel(row, col, vals, X, weights, bias):
    import ml_dtypes

    X = np.asarray(X).astype(np.float32)
    weights = np.asarray(weights).astype(np.float32)
    bias = np.asarray(bias).astype(np.float32)
    col = np.asarray(col).astype(np.int64)

    dchs, off_b, slot_idx, orders, w, owner = _preprocess(row, col, vals)
    nc = _get_program(dchs)

    tot = int(P * dchs.sum())
    wmat_bf = weights.astype(ml_dtypes.bfloat16)
    biasr = np.ascontiguousarray(bias.reshape(C, 1))

    in_maps = []
    for c in range(N_CORES):
        m = owner == c
        msgs = np.zeros((tot, C), np.float32)
        msgs[slot_idx[c]] = X[col[m]] * w[m][:, None]
        msgsT = np.ascontiguousarray(msgs.T).astype(ml_dtypes.bfloat16)
        in_maps.append({"msgs": msgsT, "wmat": wmat_bf, "biasr": biasr})

    res = _run(nc, in_maps)

    out = np.empty((N_NODES, C), np.float32)
    for c in range(N_CORES):
        y = np.asarray(res[c]["y"]).astype(np.float32)   # [C, NPC_PAD]
        out[c * NPC + orders[c]] = y.T[:NPC]
    return out


# revision 9
# speedup vs baseline: 1.8101x; 1.0632x over previous
import os
import sys

sys.path.insert(0, "/opt/trn_rl_repo")

import numpy as np

import concourse.bass as bass
import concourse.mybir as mybir
import concourse.tile as tile
from concourse import bacc

# ---------------- problem constants (hardcoded per spec) ----------------
N_NODES = 100000
N_EDGES = 640000
C = 128           # channels (in == out)
P = 128           # partitions
N_CORES = 8
NPC = N_NODES // N_CORES          # 12500 nodes per core
NBLK = (NPC + P - 1) // P         # 98 blocks per core
NPC_PAD = NBLK * P                # 12544
MAX_RUN_SLOTS = 6144              # cap per-DMA tile at 12KB/partition (bf16)

F32 = mybir.dt.float32
BF16 = mybir.dt.bfloat16


def _make_runs(dchs):
    """Split blocks into runs of consecutive equal-dch blocks, capped so one
    run's slots fit a reasonable SBUF tile. Returns (dch, b0, nblocks)."""
    runs = []
    b = 0
    while b < len(dchs):
        d = int(dchs[b])
        e = b
        slots = 0
        while e < len(dchs) and int(dchs[e]) == d \
                and slots + P * d <= MAX_RUN_SLOTS:
            slots += P * d
            e += 1
        runs.append((d, b, e - b))
        b = e
    return runs


def _build_program(dchs):
    """Single launch, transposed pipeline. Host ships channel-major messages
    msgs[c, idx(b,slab,m)] = D[row]*vals_e*D[col]*X[col_e][c] (bf16), where
    row (block b, lane m) owns slots j < dchs[b]. Within a block the dch
    128-column slabs are ordered [A0..Ap-1, B0..Bp-1, tail] with p = dch//2,
    so slab Ak holds j=2k, Bk holds j=2k+1 (tail = last odd j). Per run of
    equal-dch blocks:
      one DMA in; one wide DVE bf16 add  pt = A + B  (2x mode, pre-reduce)
    Per block:
      ps[o, m] = sum_k W.T @ pt_k (+ W.T @ tail)  (PE, f32 PSUM accumulate)
      yo[o, m] = Identity(ps + bias[o])           (Act, per-partition bias)
    One DMA out per run into y[C, NPC_PAD] (channel-major)."""
    dchs = [int(d) for d in dchs]
    s_total = sum(dchs)
    tot = P * s_total
    runs = _make_runs(dchs)
    max_slots = max(P * d * n for d, _, n in runs)
    max_pair_slots = max(P * (d // 2) * n for d, _, n in runs)
    max_rb = max(n for _, _, n in runs)

    nc = bacc.Bacc("TRN2", target_bir_lowering=False, debug=False,
                   num_devices=N_CORES)
    msgs_d = nc.dram_tensor("msgs", [P, tot], BF16, kind="ExternalInput")
    wmat_d = nc.dram_tensor("wmat", [C, C], BF16, kind="ExternalInput")
    biasr_d = nc.dram_tensor("biasr", [C, 1], F32, kind="ExternalInput")
    y_d = nc.dram_tensor("y", [C, NPC_PAD], BF16, kind="ExternalOutput")

    with tile.TileContext(nc) as tc:
        with tc.tile_pool(name="consts", bufs=1) as consts, \
             tc.tile_pool(name="mpool", bufs=4) as mpool, \
             tc.tile_pool(name="qpool", bufs=4) as qpool, \
             tc.tile_pool(name="opool", bufs=3) as opool, \
             tc.tile_pool(name="ppool", bufs=4, space="PSUM") as ppool:

            wmat_t = consts.tile([C, C], BF16)
            nc.sync.dma_start(out=wmat_t[:], in_=wmat_d[:])
            biasr_t = consts.tile([C, 1], F32)
            nc.sync.dma_start(out=biasr_t[:], in_=biasr_d[:])

            off = 0
            for dch, b0, rb in runs:
                pairs, tail = dch // 2, dch % 2
                sz = P * dch * rb
                mt = mpool.tile([P, max_slots], BF16, name="mt", tag="mt")
                nc.sync.dma_start(out=mt[:, :sz], in_=msgs_d[:, off:off + sz])
                pt = None
                if pairs:
                    mv = mt[:, :sz].rearrange("p (b s) -> p b s", s=dch * P)
                    pt = qpool.tile([P, max_pair_slots], BF16, name="pt",
                                    tag="pt")
                    with nc.allow_low_precision(
                            reason="bf16 pair pre-reduce of bf16 messages"):
                        nc.vector.tensor_tensor(
                            out=pt[:, :rb * pairs * P].rearrange(
                                "p (b s) -> p b s", s=pairs * P),
                            in0=mv[:, :, :pairs * P],
                            in1=mv[:, :, pairs * P:2 * pairs * P],
                            op=mybir.AluOpType.add)
                yo = opool.tile([P, P * max_rb], BF16, name="yo", tag="yo")
                for bi in range(rb):
                    ps = ppool.tile([P, P], F32, name="ps")
                    nmm = pairs + tail
                    i = 0
                    for k in range(pairs):
                        s0 = (bi * pairs + k) * P
                        nc.tensor.matmul(
                            out=ps[:], lhsT=wmat_t[:],
                            rhs=pt[:, s0:s0 + P],
                            start=(i == 0), stop=(i == nmm - 1))
                        i += 1
                    if tail:
                        s0 = (bi * dch + 2 * pairs) * P
                        nc.tensor.matmul(
                            out=ps[:], lhsT=wmat_t[:],
                            rhs=mt[:, s0:s0 + P],
                            start=(i == 0), stop=(i == nmm - 1))
                        i += 1
                    nc.scalar.activation(
                        out=yo[:, bi * P:(bi + 1) * P], in_=ps[:],
                        func=mybir.ActivationFunctionType.Identity,
                        bias=biasr_t[:, 0:1])
                nc.sync.dma_start(
                    out=y_d[:, (b0 * P):(b0 + rb) * P],
                    in_=yo[:, :rb * P])
                off += sz
    nc.compile()
    return nc


def _preprocess(row, col, vals):
    """Host-side routing. Returns the global per-block slot budgets plus the
    per-core (edge slot index, row permutation) needed to build the message
    stream and unpermute the output."""
    row = np.asarray(row).astype(np.int64)
    col = np.asarray(col).astype(np.int64)
    vals = np.asarray(vals).astype(np.float32)

    deg = np.bincount(row, weights=vals.astype(np.float64),
                      minlength=N_NODES)
    D = (1.0 / np.sqrt(deg + 1.0)).astype(np.float32)
    # fold both normalizations into the per-edge weight
    w = D[row] * vals * D[col]

    owner = row // NPC
    cores = []
    dch_all = np.zeros((N_CORES, NBLK), np.int64)
    for c in range(N_CORES):
        m = owner == c
        r_loc = (row[m] - c * NPC).astype(np.int64)
        indeg = np.bincount(r_loc, minlength=NPC)
        order = np.argsort(-indeg, kind="stable")    # rank -> local row
        spad = np.pad(indeg[order], (0, NPC_PAD - NPC))
        dch_all[c] = spad.reshape(NBLK, P).max(1)
        cores.append((m, r_loc, order, indeg))

    dchs = np.maximum(dch_all.max(0), 1)
    off_b = np.concatenate([[0], np.cumsum(P * dchs)])[:-1]  # [NBLK]

    slot_idx = []          # per core: slot index per (its) edge
    orders = []
    for c in range(N_CORES):
        m, r_loc, order, indeg = cores[c]
        rank_of = np.empty(NPC, np.int64)
        rank_of[order] = np.arange(NPC)
        blk = rank_of // P
        lane = rank_of % P
        # ordinal j of each edge within its row
        o_e = np.argsort(r_loc, kind="stable")
        r_s = r_loc[o_e]
        starts = np.searchsorted(r_s, np.arange(NPC))
        offs = np.arange(len(r_s)) - starts[r_s]
        # physical slab order within a block: [A0..Ap-1, B0..Bp-1, tail]
        # where pair k sums j=2k (A) and j=2k+1 (B); p = dch//2.
        d_e = dchs[blk[r_s]]
        pairs_e = d_e // 2
        slab = np.where(offs < 2 * pairs_e,
                        (offs % 2) * pairs_e + offs // 2, 2 * pairs_e)
        idx_sorted = off_b[blk[r_s]] + slab * P + lane[r_s]
        idx = np.empty(len(r_s), np.int64)
        idx[o_e] = idx_sorted
        slot_idx.append(idx)
        orders.append(order)

    return dchs, off_b, slot_idx, orders, w, owner


_CACHE = {}


def _get_program(dchs):
    key = tuple(int(d) for d in dchs)
    if key not in _CACHE:
        _CACHE[key] = _build_program(dchs)
    return _CACHE[key]


def _run(nc, in_maps):
    if os.environ.get("KERNEL_SIM"):
        from concourse import bass_interp
        sim = bass_interp.MultiCoreSim(nc, N_CORES)
        for c in range(N_CORES):
            for k, v in in_maps[c].items():
                sim.cores[c].tensor(k)[:] = v
        sim.simulate()
        out_names = [
            a.memorylocations[0].name
            for a in nc.m.functions[0].allocations
            if isinstance(a, mybir.MemoryLocationSet)
            and a.kind == "ExternalOutput"
        ]
        return [{n: np.array(sim.cores[c].mem_tensor(n)) for n in out_names}
                for c in range(N_CORES)]
    from concourse.bass_utils import run_bass_kernel_spmd
    try:
        res = run_bass_kernel_spmd(nc, in_maps, core_ids=list(range(N_CORES)))
    except Exception:
        import time
        time.sleep(2.0)  # transient NRT/axon failures recover on retry
        res = run_bass_kernel_spmd(nc, in_maps, core_ids=list(range(N_CORES)))
    return res.results


def kernel(row, col, vals, X, weights, bias):
    import ml_dtypes

    X = np.asarray(X).astype(np.float32)
    weights = np.asarray(weights).astype(np.float32)
    bias = np.asarray(bias).astype(np.float32)
    col = np.asarray(col).astype(np.int64)

    dchs, off_b, slot_idx, orders, w, owner = _preprocess(row, col, vals)
    nc = _get_program(dchs)

    tot = int(P * dchs.sum())
    wmat_bf = weights.astype(ml_dtypes.bfloat16)
    biasr = np.ascontiguousarray(bias.reshape(C, 1))

    in_maps = []
    for c in range(N_CORES):
        m = owner == c
        msgs = np.zeros((tot, C), np.float32)
        msgs[slot_idx[c]] = X[col[m]] * w[m][:, None]
        msgsT = np.ascontiguousarray(msgs.T).astype(ml_dtypes.bfloat16)
        in_maps.append({"msgs": msgsT, "wmat": wmat_bf, "biasr": biasr})

    res = _run(nc, in_maps)

    out = np.empty((N_NODES, C), np.float32)
    for c in range(N_CORES):
        y = np.asarray(res[c]["y"]).astype(np.float32)   # [C, NPC_PAD]
        out[c * NPC + orders[c]] = y.T[:NPC]
    return out


# revision 15
# speedup vs baseline: 1.8618x; 1.0286x over previous
import os
import sys

sys.path.insert(0, "/opt/trn_rl_repo")

import numpy as np

import concourse.bass as bass
import concourse.mybir as mybir
import concourse.tile as tile
from concourse import bacc

# ---------------- problem constants (hardcoded per spec) ----------------
N_NODES = 100000
N_EDGES = 640000
C = 128           # channels (in == out)
P = 128           # partitions
N_CORES = 8
NPC = N_NODES // N_CORES          # 12500 nodes per core
NBLK = (NPC + P - 1) // P         # 98 blocks per core
NPC_PAD = NBLK * P                # 12544
MAX_RUN_SLOTS = 6144              # cap per-DMA tile at 12KB/partition (bf16)

F32 = mybir.dt.float32
BF16 = mybir.dt.bfloat16


def _make_runs(dchs):
    """Split blocks into runs of consecutive equal-dch blocks, capped so one
    run's slots fit a reasonable SBUF tile. Returns (dch, b0, nblocks)."""
    runs = []
    b = 0
    while b < len(dchs):
        d = int(dchs[b])
        e = b
        slots = 0
        while e < len(dchs) and int(dchs[e]) == d \
                and slots + P * d <= MAX_RUN_SLOTS:
            slots += P * d
            e += 1
        runs.append((d, b, e - b))
        b = e
    return runs


def _stream_runs(dchs):
    """Runs in stream order: ascending dch, so Act-heavy many-block runs
    overlap with later DMAs and the compute-light biggest-dch run drains
    last. The message stream is laid out in this order."""
    return sorted(_make_runs(dchs), key=lambda r: (r[0], r[1]))


def _build_program(dchs):
    """Single launch, transposed pipeline. Host ships channel-major messages
    msgs[c, idx(b,slab,m)] = D[row]*vals_e*D[col]*X[col_e][c] (bf16), where
    row (block b, lane m) owns slots j < dchs[b]. Within a block the dch
    128-column slabs are ordered [A0..Ap-1, B0..Bp-1, tail] with p = dch//2,
    so slab Ak holds j=2k, Bk holds j=2k+1 (tail = last odd j). Per run of
    equal-dch blocks:
      one DMA in; one wide DVE bf16 add  pt = A + B  (2x mode, pre-reduce)
    Per block:
      ps[o, m] = sum_k W.T @ pt_k (+ W.T @ tail)  (PE, f32 PSUM accumulate)
      yo[o, m] = Identity(ps + bias[o])           (Act, per-partition bias)
    One DMA out per run into y[C, NPC_PAD] (channel-major)."""
    dchs = [int(d) for d in dchs]
    s_total = sum(dchs)
    tot = P * s_total
    runs = _stream_runs(dchs)
    max_slots = max(P * d * n for d, _, n in runs)
    max_pair_slots = max(P * (d // 2) * n for d, _, n in runs)
    max_rb = max(n for _, _, n in runs)

    nc = bacc.Bacc("TRN2", target_bir_lowering=False, debug=False,
                   num_devices=N_CORES)
    msgs_d = nc.dram_tensor("msgs", [P, tot], BF16, kind="ExternalInput")
    wmat_d = nc.dram_tensor("wmat", [C, C], BF16, kind="ExternalInput")
    biasr_d = nc.dram_tensor("biasr", [C, 1], F32, kind="ExternalInput")
    y_d = nc.dram_tensor("y", [C, NPC_PAD], BF16, kind="ExternalOutput")

    with tile.TileContext(nc) as tc:
        with tc.tile_pool(name="consts", bufs=1) as consts, \
             tc.tile_pool(name="mpool", bufs=6) as mpool, \
             tc.tile_pool(name="qpool", bufs=4) as qpool, \
             tc.tile_pool(name="opool", bufs=3) as opool, \
             tc.tile_pool(name="ppool", bufs=4, space="PSUM") as ppool:

            wmat_t = consts.tile([C, C], BF16)
            nc.scalar.dma_start(out=wmat_t[:], in_=wmat_d[:])
            biasr_t = consts.tile([C, 1], F32)
            nc.scalar.dma_start(out=biasr_t[:], in_=biasr_d[:])

            off = 0
            for dch, b0, rb in runs:
                pairs, tail = dch // 2, dch % 2
                sz = P * dch * rb
                mt = mpool.tile([P, max_slots], BF16, name="mt", tag="mt")
                nc.sync.dma_start(out=mt[:, :sz], in_=msgs_d[:, off:off + sz])
                pt = None
                if pairs:
                    mv = mt[:, :sz].rearrange("p (b s) -> p b s", s=dch * P)
                    pt = qpool.tile([P, max_pair_slots], BF16, name="pt",
                                    tag="pt")
                    with nc.allow_low_precision(
                            reason="bf16 pair pre-reduce of bf16 messages"):
                        nc.vector.tensor_tensor(
                            out=pt[:, :rb * pairs * P].rearrange(
                                "p (b s) -> p b s", s=pairs * P),
                            in0=mv[:, :, :pairs * P],
                            in1=mv[:, :, pairs * P:2 * pairs * P],
                            op=mybir.AluOpType.add)
                yo = opool.tile([P, P * max_rb], BF16, name="yo", tag="yo")
                for bi in range(rb):
                    ps = ppool.tile([P, P], F32, name="ps")
                    nmm = pairs + tail
                    i = 0
                    for k in range(pairs):
                        s0 = (bi * pairs + k) * P
                        nc.tensor.matmul(
                            out=ps[:], lhsT=wmat_t[:],
                            rhs=pt[:, s0:s0 + P],
                            start=(i == 0), stop=(i == nmm - 1))
                        i += 1
                    if tail:
                        s0 = (bi * dch + 2 * pairs) * P
                        nc.tensor.matmul(
                            out=ps[:], lhsT=wmat_t[:],
                            rhs=mt[:, s0:s0 + P],
                            start=(i == 0), stop=(i == nmm - 1))
                        i += 1
                    nc.scalar.activation(
                        out=yo[:, bi * P:(bi + 1) * P], in_=ps[:],
                        func=mybir.ActivationFunctionType.Identity,
                        bias=biasr_t[:, 0:1])
                nc.scalar.dma_start(
                    out=y_d[:, (b0 * P):(b0 + rb) * P],
                    in_=yo[:, :rb * P])
                off += sz
    nc.compile()
    return nc


def _preprocess(row, col, vals):
    """Host-side routing. Returns the global per-block slot budgets plus the
    per-core (edge slot index, row permutation) needed to build the message
    stream and unpermute the output."""
    row = np.asarray(row).astype(np.int64)
    col = np.asarray(col).astype(np.int64)
    vals = np.asarray(vals).astype(np.float32)

    deg = np.bincount(row, weights=vals.astype(np.float64),
                      minlength=N_NODES)
    D = (1.0 / np.sqrt(deg + 1.0)).astype(np.float32)
    # fold both normalizations into the per-edge weight
    w = D[row] * vals * D[col]

    owner = row // NPC
    cores = []
    dch_all = np.zeros((N_CORES, NBLK), np.int64)
    for c in range(N_CORES):
        m = owner == c
        r_loc = (row[m] - c * NPC).astype(np.int64)
        indeg = np.bincount(r_loc, minlength=NPC)
        order = np.argsort(-indeg, kind="stable")    # rank -> local row
        spad = np.pad(indeg[order], (0, NPC_PAD - NPC))
        dch_all[c] = spad.reshape(NBLK, P).max(1)
        cores.append((m, r_loc, order, indeg))

    dchs = np.maximum(dch_all.max(0), 1)
    # stream offsets follow the program's run order, not block order
    off_b = np.zeros(NBLK, np.int64)
    off = 0
    for d, b0, rb in _stream_runs(dchs):
        off_b[b0:b0 + rb] = off + np.arange(rb) * P * d
        off += P * d * rb

    slot_idx = []          # per core: slot index per (its) edge
    orders = []
    for c in range(N_CORES):
        m, r_loc, order, indeg = cores[c]
        rank_of = np.empty(NPC, np.int64)
        rank_of[order] = np.arange(NPC)
        blk = rank_of // P
        lane = rank_of % P
        # ordinal j of each edge within its row
        o_e = np.argsort(r_loc, kind="stable")
        r_s = r_loc[o_e]
        starts = np.searchsorted(r_s, np.arange(NPC))
        offs = np.arange(len(r_s)) - starts[r_s]
        # physical slab order within a block: [A0..Ap-1, B0..Bp-1, tail]
        # where pair k sums j=2k (A) and j=2k+1 (B); p = dch//2.
        d_e = dchs[blk[r_s]]
        pairs_e = d_e // 2
        slab = np.where(offs < 2 * pairs_e,
                        (offs % 2) * pairs_e + offs // 2, 2 * pairs_e)
        idx_sorted = off_b[blk[r_s]] + slab * P + lane[r_s]
        idx = np.empty(len(r_s), np.int64)
        idx[o_e] = idx_sorted
        slot_idx.append(idx)
        orders.append(order)

    return dchs, off_b, slot_idx, orders, w, owner


_CACHE = {}


def _get_program(dchs):
    key = tuple(int(d) for d in dchs)
    if key not in _CACHE:
        _CACHE[key] = _build_program(dchs)
    return _CACHE[key]


def _run(nc, in_maps):
    if os.environ.get("KERNEL_SIM"):
        from concourse import bass_interp
        sim = bass_interp.MultiCoreSim(nc, N_CORES)
        for c in range(N_CORES):
            for k, v in in_maps[c].items():
                sim.cores[c].tensor(k)[:] = v
        sim.simulate()
        out_names = [
            a.memorylocations[0].name
            for a in nc.m.functions[0].allocations
            if isinstance(a, mybir.MemoryLocationSet)
            and a.kind == "ExternalOutput"
        ]
        return [{n: np.array(sim.cores[c].mem_tensor(n)) for n in out_names}
                for c in range(N_CORES)]
    from concourse.bass_utils import run_bass_kernel_spmd
    try:
        res = run_bass_kernel_spmd(nc, in_maps, core_ids=list(range(N_CORES)))
    except Exception:
        import time
        time.sleep(2.0)  # transient NRT/axon failures recover on retry
        res = run_bass_kernel_spmd(nc, in_maps, core_ids=list(range(N_CORES)))
    return res.results


def kernel(row, col, vals, X, weights, bias):
    import ml_dtypes

    X = np.asarray(X).astype(np.float32)
    weights = np.asarray(weights).astype(np.float32)
    bias = np.asarray(bias).astype(np.float32)
    col = np.asarray(col).astype(np.int64)

    dchs, off_b, slot_idx, orders, w, owner = _preprocess(row, col, vals)
    nc = _get_program(dchs)

    tot = int(P * dchs.sum())
    wmat_bf = weights.astype(ml_dtypes.bfloat16)
    biasr = np.ascontiguousarray(bias.reshape(C, 1))

    in_maps = []
    for c in range(N_CORES):
        m = owner == c
        msgs = np.zeros((tot, C), np.float32)
        msgs[slot_idx[c]] = X[col[m]] * w[m][:, None]
        msgsT = np.ascontiguousarray(msgs.T).astype(ml_dtypes.bfloat16)
        in_maps.append({"msgs": msgsT, "wmat": wmat_bf, "biasr": biasr})

    res = _run(nc, in_maps)

    out = np.empty((N_NODES, C), np.float32)
    for c in range(N_CORES):
        y = np.asarray(res[c]["y"]).astype(np.float32)   # [C, NPC_PAD]
        out[c * NPC + orders[c]] = y.T[:NPC]
    return out


# revision 20
# speedup vs baseline: 1.9814x; 1.0642x over previous
import os
import sys

sys.path.insert(0, "/opt/trn_rl_repo")

import numpy as np

import concourse.bass as bass
import concourse.mybir as mybir
import concourse.tile as tile
from concourse import bacc

# ---------------- problem constants (hardcoded per spec) ----------------
N_NODES = 100000
N_EDGES = 640000
C = 128           # channels (in == out)
P = 128           # partitions
N_CORES = 8
NPC = N_NODES // N_CORES          # 12500 nodes per core
NBLK = (NPC + P - 1) // P         # 98 blocks per core
NPC_PAD = NBLK * P                # 12544
MAX_RUN_SLOTS = 6144              # cap per-DMA tile at 12KB/partition (bf16)

F32 = mybir.dt.float32
BF16 = mybir.dt.bfloat16


def _make_runs(dchs):
    """Split blocks into runs of consecutive equal-dch blocks, capped so one
    run's slots fit a reasonable SBUF tile. Returns (dch, b0, nblocks)."""
    runs = []
    b = 0
    while b < len(dchs):
        d = int(dchs[b])
        e = b
        slots = 0
        while e < len(dchs) and int(dchs[e]) == d \
                and slots + P * d <= MAX_RUN_SLOTS:
            slots += P * d
            e += 1
        runs.append((d, b, e - b))
        b = e
    return runs


def _stream_runs(dchs):
    """Runs in stream order: ascending dch, so Act-heavy many-block runs
    overlap with later DMAs and the compute-light biggest-dch run drains
    last. The message stream is laid out in this order."""
    return sorted(_make_runs(dchs), key=lambda r: (r[0], -r[1]))


GROUP_SLOTS = 12288               # input-DMA granularity (24KB/partition)


def _stream_groups(dchs):
    """Coalesce stream-consecutive runs into DMA groups. Each group gets one
    input DMA and one output DMA; runs inside keep their own DVE/PE work.
    Stream-consecutive runs cover a contiguous block range (the ascending-dch
    order reverses the descending-dch block sort), so one y slice per group
    works. Returns list of (runs, gslots, gb0, gnb)."""
    groups = []
    cur, slots = [], 0
    for d, b0, rb in _stream_runs(dchs):
        sz = P * d * rb
        if cur and slots + sz > GROUP_SLOTS:
            groups.append(cur)
            cur, slots = [], 0
        cur.append((d, b0, rb))
        slots += sz
    if cur:
        groups.append(cur)
    out = []
    for runs in groups:
        gslots = sum(P * d * rb for d, _, rb in runs)
        gb0 = min(b0 for _, b0, _ in runs)
        gnb = sum(rb for _, _, rb in runs)
        assert max(b0 + rb for _, b0, rb in runs) == gb0 + gnb
        out.append((runs, gslots, gb0, gnb))
    return out


def _build_program(dchs):
    """Single launch, transposed pipeline. Host ships channel-major messages
    msgs[c, idx(b,slab,m)] = D[row]*vals_e*D[col]*X[col_e][c] (bf16), where
    row (block b, lane m) owns slots j < dchs[b]. Within a block the dch
    128-column slabs are ordered [A0..Ap-1, B0..Bp-1, tail] with p = dch//2,
    so slab Ak holds j=2k, Bk holds j=2k+1 (tail = last odd j). Per run of
    equal-dch blocks:
      one DMA in; one wide DVE bf16 add  pt = A + B  (2x mode, pre-reduce)
    Per block:
      ps[o, m] = sum_k W.T @ pt_k (+ W.T @ tail)  (PE, f32 PSUM accumulate)
      yo[o, m] = Identity(ps + bias[o])           (Act, per-partition bias)
    One DMA out per run into y[C, NPC_PAD] (channel-major)."""
    dchs = [int(d) for d in dchs]
    s_total = sum(dchs)
    tot = P * s_total
    groups = _stream_groups(dchs)
    all_runs = [r for runs, *_ in groups for r in runs]
    max_slots = max(gslots for _, gslots, _, _ in groups)
    max_pair_slots = max(P * (d // 2) * n for d, _, n in all_runs)
    max_gnb = max(gnb for *_, gnb in groups)

    nc = bacc.Bacc("TRN2", target_bir_lowering=False, debug=False,
                   num_devices=N_CORES)
    msgs_d = nc.dram_tensor("msgs", [P, tot], BF16, kind="ExternalInput")
    wmat_d = nc.dram_tensor("wmat", [C, C], BF16, kind="ExternalInput")
    biasr_d = nc.dram_tensor("biasr", [C, 1], F32, kind="ExternalInput")
    y_d = nc.dram_tensor("y", [C, NPC_PAD], BF16, kind="ExternalOutput")

    with tile.TileContext(nc) as tc:
        with tc.tile_pool(name="consts", bufs=1) as consts, \
             tc.tile_pool(name="mpool", bufs=4) as mpool, \
             tc.tile_pool(name="qpool", bufs=4) as qpool, \
             tc.tile_pool(name="opool", bufs=3) as opool, \
             tc.tile_pool(name="ppool", bufs=4, space="PSUM") as ppool:

            wmat_t = consts.tile([C, C], BF16)
            nc.scalar.dma_start(out=wmat_t[:], in_=wmat_d[:])
            biasr_t = consts.tile([C, 1], F32)
            nc.scalar.dma_start(out=biasr_t[:], in_=biasr_d[:])

            off = 0
            for runs, gslots, gb0, gnb in groups:
                mt = mpool.tile([P, max_slots], BF16, name="mt", tag="mt")
                nc.sync.dma_start(out=mt[:, :gslots],
                                  in_=msgs_d[:, off:off + gslots])
                yo = opool.tile([P, P * max_gnb], BF16, name="yo", tag="yo")
                roff = 0
                for dch, b0, rb in runs:
                    pairs, tail = dch // 2, dch % 2
                    sz = P * dch * rb
                    pt = None
                    if pairs:
                        mv = mt[:, roff:roff + sz].rearrange(
                            "p (b s) -> p b s", s=dch * P)
                        pt = qpool.tile([P, max_pair_slots], BF16, name="pt",
                                        tag="pt")
                        with nc.allow_low_precision(
                                reason="bf16 pair pre-reduce of bf16 msgs"):
                            nc.vector.tensor_tensor(
                                out=pt[:, :rb * pairs * P].rearrange(
                                    "p (b s) -> p b s", s=pairs * P),
                                in0=mv[:, :, :pairs * P],
                                in1=mv[:, :, pairs * P:2 * pairs * P],
                                op=mybir.AluOpType.add)
                    for bi in range(rb):
                        ps = ppool.tile([P, P], F32, name="ps")
                        nmm = pairs + tail
                        i = 0
                        for k in range(pairs):
                            s0 = (bi * pairs + k) * P
                            nc.tensor.matmul(
                                out=ps[:], lhsT=wmat_t[:],
                                rhs=pt[:, s0:s0 + P],
                                start=(i == 0), stop=(i == nmm - 1))
                            i += 1
                        if tail:
                            s0 = roff + (bi * dch + 2 * pairs) * P
                            nc.tensor.matmul(
                                out=ps[:], lhsT=wmat_t[:],
                                rhs=mt[:, s0:s0 + P],
                                start=(i == 0), stop=(i == nmm - 1))
                            i += 1
                        yoff = (b0 - gb0 + bi) * P
                        nc.scalar.activation(
                            out=yo[:, yoff:yoff + P], in_=ps[:],
                            func=mybir.ActivationFunctionType.Identity,
                            bias=biasr_t[:, 0:1])
                    roff += sz
                nc.scalar.dma_start(
                    out=y_d[:, (gb0 * P):(gb0 + gnb) * P],
                    in_=yo[:, :gnb * P])
                off += gslots
    nc.compile()
    return nc


def _preprocess(row, col, vals):
    """Host-side routing. Returns the global per-block slot budgets plus the
    per-core (edge slot index, row permutation) needed to build the message
    stream and unpermute the output."""
    row = np.asarray(row).astype(np.int64)
    col = np.asarray(col).astype(np.int64)
    vals = np.asarray(vals).astype(np.float32)

    deg = np.bincount(row, weights=vals.astype(np.float64),
                      minlength=N_NODES)
    D = (1.0 / np.sqrt(deg + 1.0)).astype(np.float32)
    # fold both normalizations into the per-edge weight
    w = D[row] * vals * D[col]

    owner = row // NPC
    cores = []
    dch_all = np.zeros((N_CORES, NBLK), np.int64)
    for c in range(N_CORES):
        m = owner == c
        r_loc = (row[m] - c * NPC).astype(np.int64)
        indeg = np.bincount(r_loc, minlength=NPC)
        order = np.argsort(-indeg, kind="stable")    # rank -> local row
        spad = np.pad(indeg[order], (0, NPC_PAD - NPC))
        dch_all[c] = spad.reshape(NBLK, P).max(1)
        cores.append((m, r_loc, order, indeg))

    dchs = np.maximum(dch_all.max(0), 1)
    # stream offsets follow the program's run order, not block order
    off_b = np.zeros(NBLK, np.int64)
    off = 0
    for d, b0, rb in _stream_runs(dchs):
        off_b[b0:b0 + rb] = off + np.arange(rb) * P * d
        off += P * d * rb

    slot_idx = []          # per core: slot index per (its) edge
    orders = []
    for c in range(N_CORES):
        m, r_loc, order, indeg = cores[c]
        rank_of = np.empty(NPC, np.int64)
        rank_of[order] = np.arange(NPC)
        blk = rank_of // P
        lane = rank_of % P
        # ordinal j of each edge within its row
        o_e = np.argsort(r_loc, kind="stable")
        r_s = r_loc[o_e]
        starts = np.searchsorted(r_s, np.arange(NPC))
        offs = np.arange(len(r_s)) - starts[r_s]
        # physical slab order within a block: [A0..Ap-1, B0..Bp-1, tail]
        # where pair k sums j=2k (A) and j=2k+1 (B); p = dch//2.
        d_e = dchs[blk[r_s]]
        pairs_e = d_e // 2
        slab = np.where(offs < 2 * pairs_e,
                        (offs % 2) * pairs_e + offs // 2, 2 * pairs_e)
        idx_sorted = off_b[blk[r_s]] + slab * P + lane[r_s]
        idx = np.empty(len(r_s), np.int64)
        idx[o_e] = idx_sorted
        slot_idx.append(idx)
        orders.append(order)

    return dchs, off_b, slot_idx, orders, w, owner


_CACHE = {}


def _get_program(dchs):
    key = tuple(int(d) for d in dchs)
    if key not in _CACHE:
        _CACHE[key] = _build_program(dchs)
    return _CACHE[key]


def _run(nc, in_maps):
    if os.environ.get("KERNEL_SIM"):
        from concourse import bass_interp
        sim = bass_interp.MultiCoreSim(nc, N_CORES)
        for c in range(N_CORES):
            for k, v in in_maps[c].items():
                sim.cores[c].tensor(k)[:] = v
        sim.simulate()
        out_names = [
            a.memorylocations[0].name
            for a in nc.m.functions[0].allocations
            if isinstance(a, mybir.MemoryLocationSet)
            and a.kind == "ExternalOutput"
        ]
        return [{n: np.array(sim.cores[c].mem_tensor(n)) for n in out_names}
                for c in range(N_CORES)]
    from concourse.bass_utils import run_bass_kernel_spmd
    try:
        res = run_bass_kernel_spmd(nc, in_maps, core_ids=list(range(N_CORES)))
    except Exception:
        import time
        time.sleep(2.0)  # transient NRT/axon failures recover on retry
        res = run_bass_kernel_spmd(nc, in_maps, core_ids=list(range(N_CORES)))
    return res.results


def kernel(row, col, vals, X, weights, bias):
    import ml_dtypes

    X = np.asarray(X).astype(np.float32)
    weights = np.asarray(weights).astype(np.float32)
    bias = np.asarray(bias).astype(np.float32)
    col = np.asarray(col).astype(np.int64)

    dchs, off_b, slot_idx, orders, w, owner = _preprocess(row, col, vals)
    nc = _get_program(dchs)

    tot = int(P * dchs.sum())
    wmat_bf = weights.astype(ml_dtypes.bfloat16)
    biasr = np.ascontiguousarray(bias.reshape(C, 1))

    in_maps = []
    for c in range(N_CORES):
        m = owner == c
        msgs = np.zeros((tot, C), np.float32)
        msgs[slot_idx[c]] = X[col[m]] * w[m][:, None]
        msgsT = np.ascontiguousarray(msgs.T).astype(ml_dtypes.bfloat16)
        in_maps.append({"msgs": msgsT, "wmat": wmat_bf, "biasr": biasr})

    res = _run(nc, in_maps)

    out = np.empty((N_NODES, C), np.float32)
    for c in range(N_CORES):
        y = np.asarray(res[c]["y"]).astype(np.float32)   # [C, NPC_PAD]
        out[c * NPC + orders[c]] = y.T[:NPC]
    return out
